# revision 24
# baseline (speedup 1.0000x reference)
"""Trainium2 Bass kernel for nn_ActionDecoder (MoE-routed 2-layer GELU MLP).

Problem: per batch row b (2048 rows x 16 timesteps), route through the
embodiment_ids[b]-th expert MLP: out = GELU(x @ W1[e] + b1[e]) @ W2[e] + b2[e].
x: [2048, 16, 512] f32, W1: [4, 512, 1024], W2: [4, 1024, 28].

Strategy (expert-parallel): host sorts batch rows by embodiment, gives each of
the 8 cores one expert (2 cores per expert, half the expert's rows each). Each
core runs a dense 2-layer MLP over its tokens with its own expert's weights
(weights are per-core *data*, so one SPMD program serves all cores). Activations
are fed transposed ([d, tok]) so both matmuls keep weights stationary.

Precision: x in fp8-e3m4 as the MOVING operand (scale 2^1, dequant folded into
the GELU's scale operand), W1/W2/h in fp16. Measured on hardware: an
e3m4-moving x fp16-stationary matmul streams at the same 216ns/512-col rate
as fp16 x fp16 and is bit-exact, so fp8 x costs nothing on the PE and halves
the x DMA bytes (the head's critical path and the largest HBM stream).
End-to-end error 1.54e-2 against the 2e-2 gate. Rejected alternatives:
fp8 W1 as stationary degrades the stream to 259ns/col; e4m3 DoubleRow
streams pair-columns at the same rate so accuracy-preserving hi/lo splits
cancel the MAC gain; single-e4m3 fails the gate even with optimal linear
corrections; e3m4 DoubleRow is rejected by walrus codegen.

Perf notes:
- Token dim tiled as 512-token tiles; a 128-token remainder is re-split into
  384+256 because sizes >= 256 keep the PE at its streaming rate (128-wide
  matmuls are LDWEIGHTS-bound at ~2x the per-token cost).
- Measured DGE behavior: the sync queue starts ~1.7us after issue at
  ~170GB/s; the scalar queue lags ~3.4us; gpsimd's software queue ~90GB/s.
  The head therefore puts x0 (256KB in fp8) + w1[hc0-1] + x1 + x2 on sync in
  consumption order, w1[hc2-7] on scalar (its startup lag overlaps tile 0's
  early h-chunks), and the tiny w2/b loads on gpsimd.
- Layer 2 (M=28) packs 4 h-chunks into the 4 PE column groups concurrently
  (tile_position), then combines the 4 PSUM partition strips on DVE.
- A few dependency-free warmup matmuls run during the initial DMA wait so
  the PE p-state/HAM ramp completes before real work arrives.
"""

import os

import numpy as np
import ml_dtypes

import concourse.bacc as bacc
import concourse.mybir as mybir
from concourse.tile import TileContext
from concourse.bass_utils import run_bass_kernel_spmd

# Model dims (hardcoded per problem spec)
D = 512      # d_model
H = 1024     # hidden
A = 28       # max action dim
E = 4        # n embodiments
N_CORES = 8
P = 128      # partitions
TILE = 512   # main token tile
GRAIN = 128  # token granularity (min tile)
MIN_TILE = 256  # below this, matmuls go LDWEIGHTS-bound
KC = D // P  # 4 contraction chunks for layer 1
HC = H // P  # 8 hidden chunks

F8_MAX = 15.5  # e3m4 max finite

PS_H_BUFS = 3      # fused-gelu L1 PSUM slots (2 banks each)
PS_O_BUFS = 2      # layer-2 PSUM slots (1 bank each); ps_h*2 + ps_o <= 8
N_WARMUP_MM = 10   # spans the DMA head; p-state ramp finishes as data lands
PACK_L2 = True     # pack layer-2 into PE column groups
W1_SHIFT = 7       # e3m4 scale for the tile-0 fast-path W1 head chunks

F32 = mybir.dt.float32
F16 = mybir.dt.float16
F8 = mybir.dt.float8e3
NP_F8 = ml_dtypes.float8_e3m4

_PROGRAM_CACHE = {}

# Set by test harness to collect a profile: None | dict (filled with results)
TRACE_SINK = None


def _tile_sizes(ntok):
    n_full, rem = divmod(ntok, TILE)
    if rem == 0:
        return [TILE] * n_full
    if rem >= MIN_TILE or n_full == 0:
        return [TILE] * n_full + [rem]
    # rem == 128: split the last 640 tokens as 384 + 256
    return [TILE] * (n_full - 1) + [TILE - GRAIN, rem + GRAIN]


def _build_program(ntok, fuse_gelu, x_shift, w1_shift):
    assert ntok % GRAIN == 0
    sizes = _tile_sizes(ntok)
    descale = 1.0 / (1 << x_shift)
    descale_q = descale / (1 << w1_shift)
    nc = bacc.Bacc()

    # x is tile-blocked: tile t occupies columns [KC*off, KC*(off+size)) as
    # a [KC, size] block, so every DMA reads contiguous per-partition runs
    x_in = nc.declare_dram_parameter("x", [P, KC * ntok], F8, isOutput=False)
    w1_in = nc.declare_dram_parameter("w1", [P, HC, KC, P], F16, isOutput=False)
    # e3m4 copy of W1[hc0-1], used only for tile 0: 64KB instead of 256KB on
    # the head's critical DMA path (fp8 stationary streams ~20% slower and
    # is ~2x noisier, but only 2 of 8 h-chunks of 1 tile see it)
    w1q_in = nc.declare_dram_parameter("w1q", [P, 2, KC, P], F8, isOutput=False)
    w2_in = nc.declare_dram_parameter("w2", [P, HC, A], F16, isOutput=False)
    b1_in = nc.declare_dram_parameter("b1", [P, HC], F32, isOutput=False)
    b2_in = nc.declare_dram_parameter("b2", [A, 1], F32, isOutput=False)
    out = nc.declare_dram_parameter("out", [A, ntok], F32, isOutput=True)

    with TileContext(nc) as tc:
        with (
            tc.tile_pool(name="wpool", bufs=1) as wpool,
            tc.tile_pool(name="xpool", bufs=4) as xpool,
            tc.tile_pool(name="hpool", bufs=3) as hpool,
            tc.tile_pool(name="opool", bufs=3) as opool,
            tc.tile_pool(name="ps_h", bufs=PS_H_BUFS if fuse_gelu else 6, space="PSUM") as ps_h_pool,
            tc.tile_pool(name="ps_o", bufs=PS_O_BUFS, space="PSUM") as ps_o_pool,
        ):
            # --- PE warmup: no data deps, runs during the initial DMA wait.
            # memset on DVE (otherwise idle until the first L2 combine) so
            # the first warmup launches as soon as the PE preamble ends.
            if N_WARMUP_MM:
                warm_x = wpool.tile([P, TILE], F16)
                nc.vector.memset(warm_x, 0.0)
                warm_shape = [P, 2, TILE] if fuse_gelu else [P, TILE]
                warm_ps = ps_h_pool.tile(warm_shape, F32, tag="ps_h")
                warm_ps = warm_ps[:, 0] if fuse_gelu else warm_ps
                for _ in range(N_WARMUP_MM):
                    nc.tensor.matmul(warm_ps, warm_x[:, :P], warm_x,
                                     start=True, stop=True)

            w1_sb = wpool.tile([P, HC, KC, P], F16)
            x_sb0 = xpool.tile([P, KC, sizes[0]], F8, tag="x")
            b1_sb = wpool.tile([P, HC], F32)
            b2_sb = wpool.tile([A, 1], F32)
            w2_sb = wpool.tile([P, HC, A], F16)

            def load_x(x_sb, off, size, eng):
                a = KC * off
                eng.dma_start(
                    out=x_sb,
                    in_=x_in[:, a:a + KC * size].rearrange("p (kc n) -> p kc n", kc=KC))

            # head (time-ordered to match tile 0's consumption): sync carries
            # the critical chain w1q[hc0-1] (64KB fp8) then x0 then x1/x2;
            # scalar (3.4us startup lag) carries w1[hc2-3] and w1[hc6-7];
            # gpsimd (software queue) carries w1[hc4-5] + the tiny w2/b loads.
            w1q_sb = wpool.tile([P, 2, KC, P], F8)
            nc.sync.dma_start(out=w1q_sb, in_=w1q_in[:])
            load_x(x_sb0, 0, sizes[0], nc.sync)
            nc.scalar.dma_start(out=w1_sb[:, 2:4], in_=w1_in[:, 2:4])
            nc.scalar.dma_start(out=w1_sb[:, 6:], in_=w1_in[:, 6:])
            nc.gpsimd.dma_start(out=w1_sb[:, 4:6], in_=w1_in[:, 4:6])
            head_x = [x_sb0]
            off0 = sizes[0]
            for t in (1, 2):
                if len(sizes) > t:
                    xt = xpool.tile([P, KC, sizes[t]], F8, tag="x")
                    load_x(xt, off0, sizes[t], nc.sync)
                    head_x.append(xt)
                    off0 += sizes[t]
            # fp16 w1[hc0-1] for tiles >= 1, behind the head chain on sync
            nc.sync.dma_start(out=w1_sb[:, 0:2], in_=w1_in[:, 0:2])
            nc.gpsimd.dma_start(out=w2_sb, in_=w2_in[:])
            nc.gpsimd.dma_start(out=b1_sb, in_=b1_in[:])
            nc.gpsimd.dma_start(out=b2_sb, in_=b2_in[:])

            def emit_l2(h_sb, off, size, t, packed):
                """Layer 2: out[:, off:off+size] = W2^T h + b2."""
                o_sb = opool.tile([A, size], F32, tag="o")
                if packed:
                    # 4 h-chunks run concurrently in the 4 PE column groups,
                    # accumulating 2 rounds; strips combined on DVE (which may
                    # read at most one PSUM operand per instruction).
                    o_ps = ps_o_pool.tile([P, size], F32, tag="ps_o")
                    for r in range(2):
                        for j in range(4):
                            hc = r * 4 + j
                            nc.tensor.matmul(
                                o_ps[32 * j:32 * j + A, :],
                                w2_sb[:, hc],
                                h_sb[:, hc],
                                start=(r == 0),
                                stop=(r == 1),
                                tile_position=(0, 32 * j),
                            )
                    nc.vector.tensor_scalar_add(o_sb, o_ps[0:A], b2_sb)
                    nc.vector.tensor_add(o_sb, o_sb, o_ps[32:32 + A])
                    nc.vector.tensor_add(o_sb, o_sb, o_ps[64:64 + A])
                    nc.vector.tensor_add(o_sb, o_sb, o_ps[96:96 + A])
                else:
                    o_ps = ps_o_pool.tile([A, size], F32, tag="ps_o")
                    for hc in range(HC):
                        nc.tensor.matmul(
                            o_ps,
                            w2_sb[:, hc],
                            h_sb[:, hc],
                            start=(hc == 0),
                            stop=(hc == HC - 1),
                        )
                    if fuse_gelu:
                        # b2 == 0: PSUM->SBUF copy on ACT (idle at the tail)
                        # so the store doesn't queue behind the previous
                        # tile's DVE strip-combine on the in-order Vector
                        nc.scalar.activation(o_sb, o_ps,
                                             mybir.ActivationFunctionType.Copy)
                    else:
                        nc.vector.tensor_scalar_add(o_sb, o_ps, b2_sb)
                # alternate store queues so the final two stores issue in
                # parallel instead of serializing on one engine
                eng = nc.sync if t % 2 == 0 else nc.scalar
                eng.dma_start(out=out[:, off:off + size], in_=o_sb)

            # Layer 2 for tile t is emitted mid-way through layer 1 of tile
            # t+1 so its matmuls never wait on a just-finished GELU (PE is
            # in-order) and its DVE/store epilogue drains under compute. The
            # final tile uses unpacked L2: its single-op DVE epilogue keeps
            # the drain tail short.
            pend = None
            off = 0
            for t, size in enumerate(sizes):
                if t < len(head_x):
                    x_sb = head_x[t]
                else:
                    x_sb = xpool.tile([P, KC, size], F8, tag="x")
                    load_x(x_sb, off, size, nc.sync)

                # --- Layer 1: h = gelu((W1^T xq) * 2^-s + b1), per h-chunk ---
                h_sb = hpool.tile([P, HC, size], F16, tag="h")

                def flush_pend(pend=pend):
                    if pend is not None:
                        packed = PACK_L2 and pend[3] < len(sizes) - 1
                        emit_l2(*pend, packed)

                if fuse_gelu:
                    # b1 == 0: one ACTIVATE per pair of h-chunks (2 PSUM
                    # banks) halves ACT per-op overhead; ACT is otherwise
                    # rate-matched with PE and every hiccup stalls it.
                    # The pair tile is allocated at the full TILE width so
                    # each half starts on a 2KB PSUM bank boundary — a
                    # [P, 2, size<512] tile would put half 1 mid-bank and
                    # matmul accumulation breaks across a bank straddle.
                    for hg in range(HC // 2):
                        # tile 0's first h-chunk pair uses the small fp8 W1
                        # copy so compute starts ~2us earlier
                        fast = (t == 0 and hg == 0)
                        ps = ps_h_pool.tile([P, 2, TILE], F32, tag="ps_h")
                        for half in range(2):
                            hc = hg * 2 + half
                            w_src = w1q_sb[:, half] if fast else w1_sb[:, hc]
                            for kc in range(KC):
                                nc.tensor.matmul(
                                    ps[:, half, :size],
                                    w_src[:, kc],
                                    x_sb[:, kc],
                                    start=(kc == 0),
                                    stop=(kc == KC - 1),
                                )
                        nc.scalar.activation(
                            h_sb[:, hg * 2:hg * 2 + 2], ps[:, :, :size],
                            mybir.ActivationFunctionType.Gelu,
                            scale=descale_q if fast else descale,
                        )
                        if hg == 0:
                            # previous tile's layer 2 goes here: mid-tile so
                            # its DVE/store epilogue drains before this
                            # tile's L1 ends (shorter pipeline tail)
                            flush_pend()
                else:
                    for hc in range(HC):
                        fast = (t == 0 and hc < 2)
                        ps = ps_h_pool.tile([P, TILE], F32, tag="ps_h")
                        w_src = w1q_sb[:, hc] if fast else w1_sb[:, hc]
                        for kc in range(KC):
                            nc.tensor.matmul(
                                ps[:, :size],
                                w_src[:, kc],
                                x_sb[:, kc],
                                start=(kc == 0),
                                stop=(kc == KC - 1),
                            )
                        nc.scalar.activation(
                            h_sb[:, hc], ps[:, :size],
                            mybir.ActivationFunctionType.Gelu,
                            bias=b1_sb[:, hc:hc + 1],
                            scale=descale_q if fast else descale,
                        )
                        if hc == 1:
                            flush_pend()

                pend = (h_sb, off, size, t)
                off += size

            packed = PACK_L2 and pend[3] < len(sizes) - 1
            emit_l2(*pend, packed)

    nc.finalize()
    return nc


def kernel(pred_action_latents, W1, b1, W2, b2, embodiment_ids):
    x = np.asarray(pred_action_latents, dtype=np.float32)
    W1 = np.asarray(W1)
    b1 = np.asarray(b1)
    W2 = np.asarray(W2)
    b2 = np.asarray(b2)
    ids = np.asarray(embodiment_ids)

    B, T, _ = x.shape
    assert W1.shape[0] == E and N_CORES == 2 * E

    # --- Host-side routing/sharding ---
    order = np.argsort(ids, kind="stable")
    counts = np.bincount(ids, minlength=E)
    starts = np.concatenate([[0], np.cumsum(counts)])

    # core 2e, 2e+1 handle expert e (first/second half of its rows)
    core_rows = []
    for e in range(E):
        rows_e = order[starts[e]:starts[e + 1]]
        h1 = (len(rows_e) + 1) // 2
        core_rows.append(rows_e[:h1])
        core_rows.append(rows_e[h1:])

    max_tok = max(len(r) * T for r in core_rows)
    ntok = max(GRAIN, ((max_tok + GRAIN - 1) // GRAIN) * GRAIN)

    # x -> e3m4 at a power-of-2 scale (dequant folded into the GELU scale)
    x_absmax = float(np.abs(x).max())
    x_shift = 1
    while (1 << x_shift) * x_absmax > F8_MAX and x_shift > 0:
        x_shift -= 1

    w1_absmax = float(np.abs(np.asarray(W1, dtype=np.float32)).max())
    w1_shift = W1_SHIFT
    while (1 << w1_shift) * w1_absmax > F8_MAX and w1_shift > 0:
        w1_shift -= 1

    fuse_gelu = not np.any(b1) and not np.any(b2)
    key = (ntok, fuse_gelu, x_shift, w1_shift)
    if key not in _PROGRAM_CACHE:
        _PROGRAM_CACHE[key] = _build_program(ntok, fuse_gelu, x_shift, w1_shift)
    nc = _PROGRAM_CACHE[key]

    x_scale = float(1 << x_shift)
    in_maps = []
    for c in range(N_CORES):
        e = c // 2
        rows = core_rows[c]
        ntok_real = len(rows) * T
        xr = np.zeros((ntok, D), dtype=np.float32)
        xr[:ntok_real] = x[rows].reshape(ntok_real, D)
        xr8 = (xr * x_scale).astype(NP_F8)
        # tile-blocked [P, KC*ntok]: tile block t = [P, KC, size] with
        # (p, kc, n) = xr[off+n, kc*P+p]; contiguous per-partition runs
        blocks = []
        o = 0
        for size in _tile_sizes(ntok):
            blocks.append(
                xr8[o:o + size].reshape(size, KC, P).transpose(2, 1, 0).reshape(P, KC * size))
            o += size
        x_dev = np.ascontiguousarray(np.concatenate(blocks, axis=1))
        # [P, HC, KC, 128]: (p, hc, kc, j) = W1[e, kc*P+p, hc*P+j]
        w1_dev = np.ascontiguousarray(
            W1[e].reshape(KC, P, HC, P).transpose(1, 2, 0, 3)
        ).astype(np.float16)
        w1q_dev = np.ascontiguousarray(
            (np.asarray(W1[e], dtype=np.float32) * float(1 << w1_shift))
            .astype(NP_F8)
            .reshape(KC, P, HC, P).transpose(1, 2, 0, 3)[:, 0:2])
        w2_dev = np.ascontiguousarray(
            W2[e].reshape(HC, P, A).transpose(1, 0, 2)
        ).astype(np.float16)
        b1_dev = np.ascontiguousarray(b1[e].reshape(HC, P).T).astype(np.float32)
        b2_dev = np.ascontiguousarray(b2[e].reshape(A, 1)).astype(np.float32)
        in_maps.append({
            "x": x_dev, "w1": w1_dev, "w1q": w1q_dev, "w2": w2_dev,
            "b1": b1_dev, "b2": b2_dev,
        })

    trace = TRACE_SINK is not None
    if trace:
        os.environ.pop("BASS_NEVER_TRACE", None)
    else:
        # An ambient BASS_TRACE would route run_bass_kernel_spmd through the
        # axon NTFF hook, which needs antenv.axon_hooks (absent in fresh
        # containers) — force tracing off unless explicitly requested.
        os.environ["BASS_NEVER_TRACE"] = "1"
    res = run_bass_kernel_spmd(nc, in_maps, core_ids=list(range(N_CORES)),
                               trace=trace)
    if trace:
        TRACE_SINK["exec_time_ns"] = res.exec_time_ns
        TRACE_SINK["mean_exec_time_ns"] = res.mean_exec_time_ns
        TRACE_SINK["profile_json"] = res.profile_json

    # --- Host-side unshard ---
    out_full = np.zeros((B, T, A), dtype=np.float32)
    for c in range(N_CORES):
        rows = core_rows[c]
        if len(rows) == 0:
            continue
        o = np.asarray(res.results[c]["out"])  # [A, ntok] f32
        out_full[rows] = o[:, :len(rows) * T].T.reshape(len(rows), T, A)
    return out_full


# revision 26
# speedup vs baseline: 1.0068x; 1.0068x over previous
"""Trainium2 Bass kernel for nn_ActionDecoder (MoE-routed 2-layer GELU MLP).

Problem: per batch row b (2048 rows x 16 timesteps), route through the
embodiment_ids[b]-th expert MLP: out = GELU(x @ W1[e] + b1[e]) @ W2[e] + b2[e].
x: [2048, 16, 512] f32, W1: [4, 512, 1024], W2: [4, 1024, 28].

Strategy (expert-parallel): host sorts batch rows by embodiment, gives each of
the 8 cores one expert (2 cores per expert, half the expert's rows each). Each
core runs a dense 2-layer MLP over its tokens with its own expert's weights
(weights are per-core *data*, so one SPMD program serves all cores). Activations
are fed transposed ([d, tok]) so both matmuls keep weights stationary.

Precision: x in fp8-e3m4 as the MOVING operand (scale 2^1, dequant folded into
the GELU's scale operand), W1/W2/h in fp16. Measured on hardware: an
e3m4-moving x fp16-stationary matmul streams at the same 216ns/512-col rate
as fp16 x fp16 and is bit-exact, so fp8 x costs nothing on the PE and halves
the x DMA bytes (the head's critical path and the largest HBM stream).
End-to-end error 1.54e-2 against the 2e-2 gate. Rejected alternatives:
fp8 W1 as stationary degrades the stream to 259ns/col; e4m3 DoubleRow
streams pair-columns at the same rate so accuracy-preserving hi/lo splits
cancel the MAC gain; single-e4m3 fails the gate even with optimal linear
corrections; e3m4 DoubleRow is rejected by walrus codegen.

Perf notes:
- Token dim tiled as 512-token tiles; a 128-token remainder is re-split into
  384+256 because sizes >= 256 keep the PE at its streaming rate (128-wide
  matmuls are LDWEIGHTS-bound at ~2x the per-token cost).
- Measured DGE behavior: the sync queue starts ~1.7us after issue at
  ~170GB/s; the scalar queue lags ~3.4us; gpsimd's software queue ~90GB/s.
  The head therefore puts x0 (256KB in fp8) + w1[hc0-1] + x1 + x2 on sync in
  consumption order, w1[hc2-7] on scalar (its startup lag overlaps tile 0's
  early h-chunks), and the tiny w2/b loads on gpsimd.
- Layer 2 (M=28) packs 4 h-chunks into the 4 PE column groups concurrently
  (tile_position), then combines the 4 PSUM partition strips on DVE.
- A few dependency-free warmup matmuls run during the initial DMA wait so
  the PE p-state/HAM ramp completes before real work arrives.
"""

import os

import numpy as np
import ml_dtypes

import concourse.bacc as bacc
import concourse.mybir as mybir
from concourse.tile import TileContext
from concourse.bass_utils import run_bass_kernel_spmd

# Model dims (hardcoded per problem spec)
D = 512      # d_model
H = 1024     # hidden
A = 28       # max action dim
E = 4        # n embodiments
N_CORES = 8
P = 128      # partitions
TILE = 512   # main token tile
GRAIN = 128  # token granularity (min tile)
MIN_TILE = 256  # below this, matmuls go LDWEIGHTS-bound
KC = D // P  # 4 contraction chunks for layer 1
HC = H // P  # 8 hidden chunks

F8_MAX = 15.5  # e3m4 max finite

PS_H_BUFS = 3      # fused-gelu L1 PSUM slots (2 banks each)
PS_O_BUFS = 2      # layer-2 PSUM slots (1 bank each); ps_h*2 + ps_o <= 8
N_WARMUP_MM = 13   # spans the DMA head; p-state ramp finishes as data lands
PACK_L2 = True     # pack layer-2 into PE column groups
W1_SHIFT = 7       # e3m4 scale for the tile-0 fast-path W1 head chunks

F32 = mybir.dt.float32
F16 = mybir.dt.float16
F8 = mybir.dt.float8e3
NP_F8 = ml_dtypes.float8_e3m4

_PROGRAM_CACHE = {}

# Set by test harness to collect a profile: None | dict (filled with results)
TRACE_SINK = None


def _tile_sizes(ntok):
    n_full, rem = divmod(ntok, TILE)
    if rem == 0:
        return [TILE] * n_full
    if rem >= MIN_TILE or n_full == 0:
        return [TILE] * n_full + [rem]
    # rem == 128: split the last 640 tokens as 384 + 256
    return [TILE] * (n_full - 1) + [TILE - GRAIN, rem + GRAIN]


def _build_program(ntok, fuse_gelu, x_shift, w1_shift):
    assert ntok % GRAIN == 0
    sizes = _tile_sizes(ntok)
    descale = 1.0 / (1 << x_shift)
    descale_q = descale / (1 << w1_shift)
    nc = bacc.Bacc()

    # x is tile-blocked: tile t occupies columns [KC*off, KC*(off+size)) as
    # a [KC, size] block, so every DMA reads contiguous per-partition runs
    x_in = nc.declare_dram_parameter("x", [P, KC * ntok], F8, isOutput=False)
    w1_in = nc.declare_dram_parameter("w1", [P, HC, KC, P], F16, isOutput=False)
    # e3m4 copy of W1[hc0-1], used only for tile 0: 64KB instead of 256KB on
    # the head's critical DMA path (fp8 stationary streams ~20% slower and
    # is ~2x noisier, but only 2 of 8 h-chunks of 1 tile see it)
    w1q_in = nc.declare_dram_parameter("w1q", [P, 2, KC, P], F8, isOutput=False)
    w2_in = nc.declare_dram_parameter("w2", [P, HC, A], F16, isOutput=False)
    b1_in = nc.declare_dram_parameter("b1", [P, HC], F32, isOutput=False)
    b2_in = nc.declare_dram_parameter("b2", [A, 1], F32, isOutput=False)
    out = nc.declare_dram_parameter("out", [A, ntok], F32, isOutput=True)

    with TileContext(nc) as tc:
        with (
            tc.tile_pool(name="wpool", bufs=1) as wpool,
            tc.tile_pool(name="xpool", bufs=4) as xpool,
            tc.tile_pool(name="hpool", bufs=3) as hpool,
            tc.tile_pool(name="opool", bufs=3) as opool,
            tc.tile_pool(name="ps_h", bufs=PS_H_BUFS if fuse_gelu else 6, space="PSUM") as ps_h_pool,
            tc.tile_pool(name="ps_o", bufs=PS_O_BUFS, space="PSUM") as ps_o_pool,
        ):
            # --- PE warmup: no data deps, runs during the initial DMA wait.
            # memset on DVE (otherwise idle until the first L2 combine) so
            # the first warmup launches as soon as the PE preamble ends.
            if N_WARMUP_MM:
                warm_x = wpool.tile([P, TILE], F16)
                nc.vector.memset(warm_x, 0.0)
                warm_shape = [P, 2, TILE] if fuse_gelu else [P, TILE]
                warm_ps = ps_h_pool.tile(warm_shape, F32, tag="ps_h")
                warm_ps = warm_ps[:, 0] if fuse_gelu else warm_ps
                for _ in range(N_WARMUP_MM):
                    nc.tensor.matmul(warm_ps, warm_x[:, :P], warm_x,
                                     start=True, stop=True)

            w1_sb = wpool.tile([P, HC, KC, P], F16)
            x_sb0 = xpool.tile([P, KC, sizes[0]], F8, tag="x")
            b1_sb = wpool.tile([P, HC], F32)
            b2_sb = wpool.tile([A, 1], F32)
            w2_sb = wpool.tile([P, HC, A], F16)

            def load_x(x_sb, off, size, eng):
                a = KC * off
                eng.dma_start(
                    out=x_sb,
                    in_=x_in[:, a:a + KC * size].rearrange("p (kc n) -> p kc n", kc=KC))

            # head (time-ordered to match tile 0's consumption): sync carries
            # the critical chain w1q[hc0-1] (64KB fp8) then x0 then x1/x2;
            # scalar (3.4us startup lag) carries w1[hc2-3] and w1[hc6-7];
            # gpsimd (software queue) carries w1[hc4-5] + the tiny w2/b loads.
            w1q_sb = wpool.tile([P, 2, KC, P], F8)
            nc.sync.dma_start(out=w1q_sb, in_=w1q_in[:])
            load_x(x_sb0, 0, sizes[0], nc.sync)
            nc.scalar.dma_start(out=w1_sb[:, 2:4], in_=w1_in[:, 2:4])
            nc.scalar.dma_start(out=w1_sb[:, 6:], in_=w1_in[:, 6:])
            nc.gpsimd.dma_start(out=w1_sb[:, 4:6], in_=w1_in[:, 4:6])
            # fp16 w1[hc0-1] (tiles >= 1) ahead of the x1/x2 prefetches: the
            # head is aggregate-bandwidth-bound, so prefetch order IS the
            # critical path
            nc.sync.dma_start(out=w1_sb[:, 0:2], in_=w1_in[:, 0:2])
            head_x = [x_sb0]
            off0 = sizes[0]
            for t in (1, 2):
                if len(sizes) > t:
                    xt = xpool.tile([P, KC, sizes[t]], F8, tag="x")
                    load_x(xt, off0, sizes[t], nc.sync)
                    head_x.append(xt)
                    off0 += sizes[t]
            nc.gpsimd.dma_start(out=w2_sb, in_=w2_in[:])
            nc.gpsimd.dma_start(out=b1_sb, in_=b1_in[:])
            nc.gpsimd.dma_start(out=b2_sb, in_=b2_in[:])

            def emit_l2(h_sb, off, size, t, packed):
                """Layer 2: out[:, off:off+size] = W2^T h + b2."""
                o_sb = opool.tile([A, size], F32, tag="o")
                if packed:
                    # 4 h-chunks run concurrently in the 4 PE column groups,
                    # accumulating 2 rounds; strips combined on DVE (which may
                    # read at most one PSUM operand per instruction).
                    o_ps = ps_o_pool.tile([P, size], F32, tag="ps_o")
                    for r in range(2):
                        for j in range(4):
                            hc = r * 4 + j
                            nc.tensor.matmul(
                                o_ps[32 * j:32 * j + A, :],
                                w2_sb[:, hc],
                                h_sb[:, hc],
                                start=(r == 0),
                                stop=(r == 1),
                                tile_position=(0, 32 * j),
                            )
                    nc.vector.tensor_scalar_add(o_sb, o_ps[0:A], b2_sb)
                    nc.vector.tensor_add(o_sb, o_sb, o_ps[32:32 + A])
                    nc.vector.tensor_add(o_sb, o_sb, o_ps[64:64 + A])
                    nc.vector.tensor_add(o_sb, o_sb, o_ps[96:96 + A])
                else:
                    o_ps = ps_o_pool.tile([A, size], F32, tag="ps_o")
                    for hc in range(HC):
                        nc.tensor.matmul(
                            o_ps,
                            w2_sb[:, hc],
                            h_sb[:, hc],
                            start=(hc == 0),
                            stop=(hc == HC - 1),
                        )
                    if fuse_gelu:
                        # b2 == 0: PSUM->SBUF copy on ACT (idle at the tail)
                        # so the store doesn't queue behind the previous
                        # tile's DVE strip-combine on the in-order Vector
                        nc.scalar.activation(o_sb, o_ps,
                                             mybir.ActivationFunctionType.Copy)
                    else:
                        nc.vector.tensor_scalar_add(o_sb, o_ps, b2_sb)
                # alternate store queues so the final two stores issue in
                # parallel instead of serializing on one engine
                eng = nc.sync if t % 2 == 0 else nc.scalar
                eng.dma_start(out=out[:, off:off + size], in_=o_sb)

            # Layer 2 for tile t is emitted mid-way through layer 1 of tile
            # t+1 so its matmuls never wait on a just-finished GELU (PE is
            # in-order) and its DVE/store epilogue drains under compute. The
            # final tile uses unpacked L2: its single-op DVE epilogue keeps
            # the drain tail short.
            pend = None
            off = 0
            for t, size in enumerate(sizes):
                if t < len(head_x):
                    x_sb = head_x[t]
                else:
                    x_sb = xpool.tile([P, KC, size], F8, tag="x")
                    load_x(x_sb, off, size, nc.sync)

                # --- Layer 1: h = gelu((W1^T xq) * 2^-s + b1), per h-chunk ---
                h_sb = hpool.tile([P, HC, size], F16, tag="h")

                def flush_pend(pend=pend):
                    if pend is not None:
                        packed = PACK_L2 and pend[3] < len(sizes) - 1
                        emit_l2(*pend, packed)

                if fuse_gelu:
                    # b1 == 0: one ACTIVATE per pair of h-chunks (2 PSUM
                    # banks) halves ACT per-op overhead; ACT is otherwise
                    # rate-matched with PE and every hiccup stalls it.
                    # The pair tile is allocated at the full TILE width so
                    # each half starts on a 2KB PSUM bank boundary — a
                    # [P, 2, size<512] tile would put half 1 mid-bank and
                    # matmul accumulation breaks across a bank straddle.
                    for hg in range(HC // 2):
                        # tile 0's first h-chunk pair uses the small fp8 W1
                        # copy so compute starts ~2us earlier
                        fast = (t == 0 and hg == 0)
                        ps = ps_h_pool.tile([P, 2, TILE], F32, tag="ps_h")
                        for half in range(2):
                            hc = hg * 2 + half
                            w_src = w1q_sb[:, half] if fast else w1_sb[:, hc]
                            for kc in range(KC):
                                nc.tensor.matmul(
                                    ps[:, half, :size],
                                    w_src[:, kc],
                                    x_sb[:, kc],
                                    start=(kc == 0),
                                    stop=(kc == KC - 1),
                                )
                        nc.scalar.activation(
                            h_sb[:, hg * 2:hg * 2 + 2], ps[:, :, :size],
                            mybir.ActivationFunctionType.Gelu,
                            scale=descale_q if fast else descale,
                        )
                        if hg == 0:
                            # previous tile's layer 2 goes here: mid-tile so
                            # its DVE/store epilogue drains before this
                            # tile's L1 ends (shorter pipeline tail)
                            flush_pend()
                else:
                    for hc in range(HC):
                        fast = (t == 0 and hc < 2)
                        ps = ps_h_pool.tile([P, TILE], F32, tag="ps_h")
                        w_src = w1q_sb[:, hc] if fast else w1_sb[:, hc]
                        for kc in range(KC):
                            nc.tensor.matmul(
                                ps[:, :size],
                                w_src[:, kc],
                                x_sb[:, kc],
                                start=(kc == 0),
                                stop=(kc == KC - 1),
                            )
                        nc.scalar.activation(
                            h_sb[:, hc], ps[:, :size],
                            mybir.ActivationFunctionType.Gelu,
                            bias=b1_sb[:, hc:hc + 1],
                            scale=descale_q if fast else descale,
                        )
                        if hc == 1:
                            flush_pend()

                pend = (h_sb, off, size, t)
                off += size

            packed = PACK_L2 and pend[3] < len(sizes) - 1
            emit_l2(*pend, packed)

    nc.finalize()
    return nc


def kernel(pred_action_latents, W1, b1, W2, b2, embodiment_ids):
    x = np.asarray(pred_action_latents, dtype=np.float32)
    W1 = np.asarray(W1)
    b1 = np.asarray(b1)
    W2 = np.asarray(W2)
    b2 = np.asarray(b2)
    ids = np.asarray(embodiment_ids)

    B, T, _ = x.shape
    assert W1.shape[0] == E and N_CORES == 2 * E

    # --- Host-side routing/sharding ---
    order = np.argsort(ids, kind="stable")
    counts = np.bincount(ids, minlength=E)
    starts = np.concatenate([[0], np.cumsum(counts)])

    # core 2e, 2e+1 handle expert e (first/second half of its rows)
    core_rows = []
    for e in range(E):
        rows_e = order[starts[e]:starts[e + 1]]
        h1 = (len(rows_e) + 1) // 2
        core_rows.append(rows_e[:h1])
        core_rows.append(rows_e[h1:])

    max_tok = max(len(r) * T for r in core_rows)
    ntok = max(GRAIN, ((max_tok + GRAIN - 1) // GRAIN) * GRAIN)

    # x -> e3m4 at a power-of-2 scale (dequant folded into the GELU scale)
    x_absmax = float(np.abs(x).max())
    x_shift = 1
    while (1 << x_shift) * x_absmax > F8_MAX and x_shift > 0:
        x_shift -= 1

    w1_absmax = float(np.abs(np.asarray(W1, dtype=np.float32)).max())
    w1_shift = W1_SHIFT
    while (1 << w1_shift) * w1_absmax > F8_MAX and w1_shift > 0:
        w1_shift -= 1

    fuse_gelu = not np.any(b1) and not np.any(b2)
    key = (ntok, fuse_gelu, x_shift, w1_shift)
    if key not in _PROGRAM_CACHE:
        _PROGRAM_CACHE[key] = _build_program(ntok, fuse_gelu, x_shift, w1_shift)
    nc = _PROGRAM_CACHE[key]

    x_scale = float(1 << x_shift)
    in_maps = []
    for c in range(N_CORES):
        e = c // 2
        rows = core_rows[c]
        ntok_real = len(rows) * T
        xr = np.zeros((ntok, D), dtype=np.float32)
        xr[:ntok_real] = x[rows].reshape(ntok_real, D)
        xr8 = (xr * x_scale).astype(NP_F8)
        # tile-blocked [P, KC*ntok]: tile block t = [P, KC, size] with
        # (p, kc, n) = xr[off+n, kc*P+p]; contiguous per-partition runs
        blocks = []
        o = 0
        for size in _tile_sizes(ntok):
            blocks.append(
                xr8[o:o + size].reshape(size, KC, P).transpose(2, 1, 0).reshape(P, KC * size))
            o += size
        x_dev = np.ascontiguousarray(np.concatenate(blocks, axis=1))
        # [P, HC, KC, 128]: (p, hc, kc, j) = W1[e, kc*P+p, hc*P+j]
        w1_dev = np.ascontiguousarray(
            W1[e].reshape(KC, P, HC, P).transpose(1, 2, 0, 3)
        ).astype(np.float16)
        w1q_dev = np.ascontiguousarray(
            (np.asarray(W1[e], dtype=np.float32) * float(1 << w1_shift))
            .astype(NP_F8)
            .reshape(KC, P, HC, P).transpose(1, 2, 0, 3)[:, 0:2])
        w2_dev = np.ascontiguousarray(
            W2[e].reshape(HC, P, A).transpose(1, 0, 2)
        ).astype(np.float16)
        b1_dev = np.ascontiguousarray(b1[e].reshape(HC, P).T).astype(np.float32)
        b2_dev = np.ascontiguousarray(b2[e].reshape(A, 1)).astype(np.float32)
        in_maps.append({
            "x": x_dev, "w1": w1_dev, "w1q": w1q_dev, "w2": w2_dev,
            "b1": b1_dev, "b2": b2_dev,
        })

    trace = TRACE_SINK is not None
    if trace:
        os.environ.pop("BASS_NEVER_TRACE", None)
    else:
        # An ambient BASS_TRACE would route run_bass_kernel_spmd through the
        # axon NTFF hook, which needs antenv.axon_hooks (absent in fresh
        # containers) — force tracing off unless explicitly requested.
        os.environ["BASS_NEVER_TRACE"] = "1"
    res = run_bass_kernel_spmd(nc, in_maps, core_ids=list(range(N_CORES)),
                               trace=trace)
    if trace:
        TRACE_SINK["exec_time_ns"] = res.exec_time_ns
        TRACE_SINK["mean_exec_time_ns"] = res.mean_exec_time_ns
        TRACE_SINK["profile_json"] = res.profile_json

    # --- Host-side unshard ---
    out_full = np.zeros((B, T, A), dtype=np.float32)
    for c in range(N_CORES):
        rows = core_rows[c]
        if len(rows) == 0:
            continue
        o = np.asarray(res.results[c]["out"])  # [A, ntok] f32
        out_full[rows] = o[:, :len(rows) * T].T.reshape(len(rows), T, A)
    return out_full
